# revision 18
# baseline (speedup 1.0000x reference)
"""AttnWindowPool Trainium2 kernel.

Math (per output row t, window w = t-3..t, per head h):
  k = e @ Wk, v = e @ Wv                       [L, H*128]
  s_h[t'] = q_h . k[t', h, :]                  (logits, shared across the 4 offsets)
  att = softmax over the window of s  ->  pooled = sum_w att_w * v[t-3+w]
Softmax-without-max identity (logits are O(5), exp can't overflow in fp32):
  pooled[t] = (sum_{s=t-3..t} E[s] * v[s]) / (sum_{s=t-3..t} E[s]),  E = exp(scale*s)
which turns the windowed attention into ONE constant banded matmul over
U[s] = E[s]*v[s] (plus the same banded sum of E for the denominator).
  out = pooled @ Wo + bo

Sharding: L split across 8 cores (2048 rows each) + one extra 128-row "halo"
block in front (previous core's last rows; zeros + E-mask for core 0).
All matmuls run as float32r (FP22 single-pass mode).
"""

import sys

sys.path.insert(0, "/opt/trn_rl_repo")

from contextlib import ExitStack

import numpy as np

import concourse.bass as bass
import concourse.tile as tile
from concourse import mybir
from concourse.masks import make_identity

# ---- problem constants (hardcoded per the grading contract) ----
L, D_IN, D_OUT, H, DH = 16384, 2048, 2048, 16, 128
W_LEFT = 3
N_CORES = 8
LC = L // N_CORES          # 2048 own rows per core
LH = LC + 128              # + one 128-row halo block in front
TB = LH // 128             # 17 t-blocks (block 0 = halo)
KC = D_IN // 128           # 16 contraction chunks
NC_ = D_OUT // 128         # 16 feature chunks
SCALE = 1.0 / float(np.sqrt(DH))
F32 = mybir.dt.float32
F32R = mybir.dt.float32r
# free-dim panels over the LH columns (matmul moving-operand max 512 fp32)
PANELS = [(0, 512), (512, 512), (1024, 512), (1536, 512), (2048, 128)]
NP4 = 4                    # 512-wide output panels over D_OUT
SPLIT_WAITS = True         # HW codegen needs 1-wait instrs; CoreSim can't parse NoOps


def _band_consts():
    d = np.zeros((128, 128), np.float32)   # diagT[s,t] = 1 if 0 <= t-s <= 3
    c = np.zeros((128, 128), np.float32)   # cornT[s,t] = 1 if s-t >= 125
    s = np.arange(128)[:, None]
    t = np.arange(128)[None, :]
    d[(t - s >= 0) & (t - s <= 3)] = 1.0
    c[(s - t) >= 125] = 1.0
    return d, c


def build_nc():
    nc = bass.Bass("TRN2", target_bir_lowering=False, debug=False)

    e_t = nc.dram_tensor("e", [LH, D_IN], F32R, kind="ExternalInput")
    emask_t = nc.dram_tensor("emask", [LH], F32, kind="ExternalInput")
    wqf_t = nc.dram_tensor("wqf", [D_IN, H], F32R, kind="ExternalInput")
    wv_t = nc.dram_tensor("wv", [D_IN, D_OUT], F32R, kind="ExternalInput")
    wo_t = nc.dram_tensor("wo", [D_OUT, D_OUT], F32R, kind="ExternalInput")
    bo_t = nc.dram_tensor("bo", [D_OUT], F32, kind="ExternalInput")
    out_t = nc.dram_tensor("out", [LC, D_OUT], F32, kind="ExternalOutput")

    vnat_t = nc.dram_tensor("vnat", [LH, D_OUT], F32R)  # scratch

    diag_np, corn_np = _band_consts()
    diag_d = nc.inline_tensor(diag_np, "band_diag")
    corn_d = nc.inline_tensor(corn_np, "band_corn")

    with tile.TileContext(nc) as tc:
        _kernel_body(
            tc, e_t, emask_t, wqf_t, wv_t, wo_t, bo_t, out_t, vnat_t,
            diag_d, corn_d,
        )
    if SPLIT_WAITS:
        _split_matmul_waits(nc)
    return nc


def _split_matmul_waits(nc):
    """walrus's LDWEIGHTS struct only has room for one sync-wait command, so
    a Matmult carrying >1 on_wait fails codegen ("Too many sync wait
    commands").  Move the extra waits onto NoOps just before the matmul on
    the PE queue (one wait per NoOp)."""
    for func in nc.m.functions:
        for blk in func.blocks:
            new_insts = []
            for inst in blk.instructions:
                si = getattr(inst, "sync_info", None)
                if (
                    si is not None
                    and si.on_wait
                    and len(si.on_wait) > 1
                    and not isinstance(inst, mybir.InstNoOp)
                ):
                    waits = list(si.on_wait)
                    for w in waits[:-1]:
                        nop = mybir.InstNoOp(
                            name=nc.get_next_instruction_name(),
                            ins=[],
                            outs=[],
                            sync_info=mybir.SyncInfo(on_wait=[w], on_update=[]),
                            bass_nofuse=True,
                            engine=inst.engine,
                        )
                        new_insts.append(nop)
                    inst.sync_info = mybir.SyncInfo(
                        on_wait=[waits[-1]], on_update=list(si.on_update)
                    )
                new_insts.append(inst)
            blk.instructions[:] = new_insts


def _kernel_body(tc, e_t, emask_t, wqf_t, wv_t, wo_t, bo_t, out_t, vnat_t,
                 diag_d, corn_d):
    nc = tc.nc
    R = lambda ap: ap.bitcast(F32R)

    with ExitStack() as top:
        # pools that live for the whole kernel
        singles = top.enter_context(tc.tile_pool(name="singles", bufs=1))
        ps_acc = top.enter_context(tc.tile_pool(name="ps_acc", bufs=1, space="PSUM"))
        ps_tr = top.enter_context(tc.tile_pool(name="ps_tr", bufs=2, space="PSUM"))

        ident0 = singles.tile([128, 128], F32)
        make_identity(nc, ident0)
        ident = singles.tile([128, 128], F32R)
        nc.vector.tensor_copy(ident[:], ident0[:])
        diag_sb = singles.tile([128, 128], F32R)
        nc.sync.dma_start(diag_sb[:], diag_d.ap().bitcast(F32R))
        corn_sb = singles.tile([128, 128], F32R)
        nc.sync.dma_start(corn_sb[:], corn_d.ap().bitcast(F32R))

        # folded logit weights Wk @ blockdiag(q): [D_IN, H]
        wqf_sb = singles.tile([128, KC, H], F32R)
        nc.sync.dma_start(
            wqf_sb[:], wqf_t.ap().rearrange("(a p) h -> p a h", p=128))

        emask_sb = singles.tile([128, TB], F32)
        nc.sync.dma_start(emask_sb[:], emask_t.ap().rearrange("(a p) -> p a", p=128))

        # E16 natural [128 t, tb, h]
        e16 = singles.tile([128, TB, H], F32)

        # ---------------- Phase A: e -> eT (PE transpose) ----------------
        pab = top.enter_context(ExitStack())
        with ExitStack() as pa:
            eT_pool = pab.enter_context(tc.tile_pool(name="eT", bufs=1))
            eblk_pool = pa.enter_context(tc.tile_pool(name="eblk", bufs=3))
            eT = eT_pool.tile([128, KC, LH], F32R)
            for tb in range(TB):
                eb = eblk_pool.tile([128, D_IN], F32R, tag="eblk")
                nc.sync.dma_start(eb[:], e_t[tb * 128 : (tb + 1) * 128, :])
                for kc in range(KC):
                    tp = ps_tr.tile([128, 128], F32R, tag="tr")
                    nc.tensor.transpose(R(tp[:]), R(eb[:, kc * 128 : (kc + 1) * 128]), R(ident[:]))
                    dst = eT[:, kc, tb * 128 : (tb + 1) * 128]
                    if (tb * KC + kc) % 2 == 0:
                        nc.vector.tensor_copy(dst, tp[:])
                    else:
                        nc.scalar.copy(dst, tp[:])

        # ---- Phase A2: logits S16[t, h] = eT.T @ wqf, E = exp(scale*S) ----
        for tb in range(TB):
            sps = ps_acc.tile([128, H], F32, tag="s16", name="sps")
            for kc in range(KC):
                nc.tensor.matmul(
                    sps[:], R(eT[:, kc, tb * 128 : (tb + 1) * 128]), R(wqf_sb[:, kc, :]),
                    start=(kc == 0), stop=(kc == KC - 1),
                )
            nc.scalar.activation(e16[:, tb, :], sps[:],
                                 mybir.ActivationFunctionType.Exp, scale=SCALE)
        nc.vector.tensor_tensor(
            e16[:], e16[:],
            emask_sb[:, :, None].to_broadcast((128, TB, H)), mybir.AluOpType.mult,
        )

        # ------------- Phase B: V projection (weights streamed once) -------------
        with ExitStack() as pb:
            wsl_pool = pb.enter_context(tc.tile_pool(name="wsl", bufs=2))
            kv_pool = pb.enter_context(tc.tile_pool(name="kv", bufs=2))
            vb_pool = pb.enter_context(tc.tile_pool(name="vb", bufs=4))
            for c in range(NC_):
                # --- V chunk: VT_c = Wv[:, c].T @ eT  -> transpose -> vnat dram
                wv_sl = wsl_pool.tile([128, KC, 128], F32R, tag="w")
                nc.sync.dma_start(
                    wv_sl[:],
                    wv_t[:, c * 128 : (c + 1) * 128].rearrange("(a p) n -> p a n", p=128),
                )
                vtc = kv_pool.tile([128, LH], F32R, tag="kv")
                accs = [
                    ps_acc.tile([128, w], F32, tag=f"acc{i}", name=f"acc{i}")
                    for i, (o, w) in enumerate(PANELS)
                ]
                for kc in range(KC):
                    for i, (o, w) in enumerate(PANELS):
                        nc.tensor.matmul(
                            accs[i][:], R(wv_sl[:, kc, :]), R(eT[:, kc, o : o + w]),
                            start=(kc == 0), stop=(kc == KC - 1),
                        )
                for i, (o, w) in enumerate(PANELS):
                    if i % 2 == 0:
                        nc.vector.tensor_copy(vtc[:, o : o + w], accs[i][:])
                    else:
                        nc.scalar.copy(vtc[:, o : o + w], accs[i][:])
                for tb in range(TB):
                    tp = ps_tr.tile([128, 128], F32R, tag="tr")
                    nc.tensor.transpose(R(tp[:]), R(vtc[:, tb * 128 : (tb + 1) * 128]), R(ident[:]))
                    vb = vb_pool.tile([128, 128], F32R, tag="vb")
                    if tb % 2 == 0:
                        nc.vector.tensor_copy(vb[:], tp[:])
                    else:
                        nc.scalar.copy(vb[:], tp[:])
                    nc.sync.dma_start(
                        vnat_t[tb * 128 : (tb + 1) * 128, c * 128 : (c + 1) * 128], vb[:]
                    )

        pab.close()  # frees eT before pooledT is allocated

        # ------------- Phase C: U = E*V, banded sums, divide, transpose -------------
        pooledT_pool = top.enter_context(tc.tile_pool(name="pooledT", bufs=1))
        pooledT = pooledT_pool.tile([128, NC_, LC], F32R)
        with ExitStack() as pc:
            vblk_pool = pc.enter_context(tc.tile_pool(name="vblk", bufs=3))
            u_pool = pc.enter_context(tc.tile_pool(name="u", bufs=2))
            pl_pool = pc.enter_context(tc.tile_pool(name="pl", bufs=2))
            sm_pool = pc.enter_context(tc.tile_pool(name="sm", bufs=2))
            u_prev = None
            for tb in range(TB):
                vb = vblk_pool.tile([128, D_OUT], F32R, tag="vblk")
                nc.sync.dma_start(vb[:], vnat_t[tb * 128 : (tb + 1) * 128, :])
                u = u_pool.tile([128, D_OUT + H], F32R, tag="u")
                nc.vector.tensor_tensor(
                    u[:, :D_OUT].rearrange("p (h d) -> p h d", d=DH),
                    vb[:].rearrange("p (h d) -> p h d", d=DH),
                    e16[:, tb, :, None].to_broadcast((128, H, DH)),
                    mybir.AluOpType.mult,
                )
                nc.scalar.copy(u[:, D_OUT:], e16[:, tb, :])
                if tb >= 1:
                    t_own = tb - 1  # output t-block index (0..15)
                    prs = []
                    for i in range(NP4):
                        pr = ps_acc.tile([128, 512], F32, tag=f"acc{i}")
                        nc.tensor.matmul(pr[:], R(diag_sb[:]), R(u[:, i * 512 : (i + 1) * 512]),
                                         start=True, stop=False)
                        nc.tensor.matmul(pr[:], R(corn_sb[:]),
                                         R(u_prev[:, i * 512 : (i + 1) * 512]),
                                         start=False, stop=True)
                        prs.append(pr)
                    prd = ps_acc.tile([128, H], F32, tag="acc4")
                    nc.tensor.matmul(prd[:], R(diag_sb[:]), R(u[:, D_OUT:]),
                                     start=True, stop=False)
                    nc.tensor.matmul(prd[:], R(corn_sb[:]), R(u_prev[:, D_OUT:]),
                                     start=False, stop=True)
                    dinv = sm_pool.tile([128, H], F32, tag="dinv")
                    nc.vector.reciprocal(dinv[:], prd[:])
                    pooled = pl_pool.tile([128, D_OUT], F32R, tag="pl")
                    for i in range(NP4):
                        nc.vector.tensor_tensor(
                            pooled[:, i * 512 : (i + 1) * 512].rearrange(
                                "p (h d) -> p h d", d=DH),
                            prs[i][:].rearrange("p (h d) -> p h d", d=DH),
                            dinv[:, i * 4 : (i + 1) * 4, None].to_broadcast(
                                (128, 4, DH)),
                            mybir.AluOpType.mult,
                        )
                    for hc in range(NC_):
                        tp = ps_tr.tile([128, 128], F32R, tag="tr")
                        nc.tensor.transpose(
                            R(tp[:]), R(pooled[:, hc * 128 : (hc + 1) * 128]), R(ident[:]))
                        dst = pooledT[:, hc, t_own * 128 : (t_own + 1) * 128]
                        if hc % 2 == 0:
                            nc.vector.tensor_copy(dst, tp[:])
                        else:
                            nc.scalar.copy(dst, tp[:])
                u_prev = u

        # ------------- Phase D: out = pooledT.T @ Wo + bo -------------
        with ExitStack() as pd:
            wo_pool = pd.enter_context(tc.tile_pool(name="wo", bufs=2))
            ob_pool = pd.enter_context(tc.tile_pool(name="ob", bufs=4))
            bo_pool = pd.enter_context(tc.tile_pool(name="bo", bufs=1))
            bo_sb = bo_pool.tile([128, D_OUT], F32)
            bo_bc = bass.AP(tensor=bo_t, offset=0, ap=[[0, 128], [1, D_OUT]])
            nc.gpsimd.dma_start(bo_sb[:], bo_bc)
            PW = 256
            for i in range(D_OUT // PW):
                wo_sl = wo_pool.tile([128, KC, PW], F32R, tag="wo")
                nc.sync.dma_start(
                    wo_sl[:],
                    wo_t[:, i * PW : (i + 1) * PW].rearrange(
                        "(a p) n -> p a n", p=128),
                )
                for tb in range(TB - 1):
                    ops = ps_acc.tile([128, PW], F32, tag=f"acc{tb % 4}",
                                      name="ops")
                    for hc in range(NC_):
                        nc.tensor.matmul(
                            ops[:], R(pooledT[:, hc, tb * 128 : (tb + 1) * 128]),
                            R(wo_sl[:, hc, :]),
                            start=(hc == 0), stop=(hc == NC_ - 1),
                        )
                    osb = ob_pool.tile([128, PW], F32, tag="ob")
                    nc.vector.tensor_tensor(
                        osb[:], ops[:], bo_sb[:, i * PW : (i + 1) * PW],
                        mybir.AluOpType.add,
                    )
                    nc.sync.dma_start(
                        out_t[tb * 128 : (tb + 1) * 128, i * PW : (i + 1) * PW],
                        osb[:],
                    )


_NC_CACHE = None


def _get_nc():
    global _NC_CACHE
    if _NC_CACHE is None:
        _NC_CACHE = build_nc()
    return _NC_CACHE


def make_in_maps(e_seq, q_param, Wk, Wv, Wo, bo):
    e_seq = np.ascontiguousarray(e_seq, np.float32)
    # fold Wk and q into the per-head logit weights (weight preprocessing):
    # s_h[t] = q_h . (e[t] @ Wk)[h*128:(h+1)*128] = e[t] . wqf[:, h]
    wqf = np.einsum(
        "khd,hd->kh",
        np.asarray(Wk, np.float32).reshape(D_IN, H, DH),
        np.asarray(q_param, np.float32),
    ).astype(np.float32)
    in_maps = []
    for c in range(N_CORES):
        s = c * LC
        if c == 0:
            eh = np.concatenate(
                [np.zeros((128, D_IN), np.float32), e_seq[:LC]], axis=0)
            em = np.ones(LH, np.float32)
            em[:128] = 0.0
        else:
            eh = e_seq[s - 128 : s + LC]
            em = np.ones(LH, np.float32)
        in_maps.append({
            "e": np.ascontiguousarray(eh),
            "emask": em,
            "wqf": wqf,
            "wv": np.ascontiguousarray(Wv, np.float32),
            "wo": np.ascontiguousarray(Wo, np.float32),
            "bo": np.ascontiguousarray(bo, np.float32),
        })
    return in_maps


_RUNNER = None


def _get_runner():
    """Cached jitted 8-core runner (mirrors bass2jax.run_bass_via_pjrt)."""
    global _RUNNER
    if _RUNNER is not None:
        return _RUNNER
    import jax
    from jax.sharding import Mesh, PartitionSpec
    from jax.experimental.shard_map import shard_map
    from concourse import mybir as mb
    from concourse.bass2jax import (
        _bass_exec_p, install_neuronx_cc_hook, partition_id_tensor,
    )

    install_neuronx_cc_hook()
    nc = _get_nc()
    partition_name = (
        nc.partition_id_tensor.name if nc.partition_id_tensor else None
    )
    in_names, out_names, out_avals, zero_shapes = [], [], [], []
    for alloc in nc.m.functions[0].allocations:
        if not isinstance(alloc, mb.MemoryLocationSet):
            continue
        name = alloc.memorylocations[0].name
        if alloc.kind == "ExternalInput":
            if name != partition_name:
                in_names.append(name)
        elif alloc.kind == "ExternalOutput":
            out_names.append(name)
            shape = tuple(alloc.tensor_shape)
            dtype = mb.dt.np(alloc.dtype)
            out_avals.append(jax.core.ShapedArray(shape, dtype))
            zero_shapes.append((shape, dtype))
    n_params = len(in_names)
    n_outs = len(out_avals)
    all_names = in_names + out_names
    if partition_name is not None:
        all_names = all_names + [partition_name]
    donate = tuple(range(n_params, n_params + n_outs))

    def _body(*args):
        operands = list(args)
        if partition_name is not None:
            operands.append(partition_id_tensor())
        outs = _bass_exec_p.bind(
            *operands,
            out_avals=tuple(out_avals),
            in_names=tuple(all_names),
            out_names=tuple(out_names),
            lowering_input_output_aliases=(),
            sim_require_finite=True,
            sim_require_nnan=True,
            nc=nc,
        )
        return tuple(outs)

    devices = jax.devices()[:N_CORES]
    mesh = Mesh(np.asarray(devices), ("core",))
    sharded = jax.jit(
        shard_map(_body, mesh=mesh,
                  in_specs=(PartitionSpec("core"),) * (n_params + n_outs),
                  out_specs=(PartitionSpec("core"),) * n_outs,
                  check_rep=False),
        donate_argnums=donate, keep_unused=True,
    )

    def runner(in_maps):
        per_core = [[np.asarray(m[n]) for n in in_names] for m in in_maps]
        concat_in = [
            np.concatenate([per_core[c][i] for c in range(N_CORES)], axis=0)
            for i in range(n_params)
        ]
        concat_zeros = [
            np.zeros((N_CORES * s[0], *s[1:]), d) for (s, d) in zero_shapes
        ]
        out_arrs = sharded(*concat_in, *concat_zeros)
        out_arrs = [np.asarray(a) for a in out_arrs]
        return [
            {n: out_arrs[i].reshape(N_CORES, *zero_shapes[i][0])[c]
             for i, n in enumerate(out_names)}
            for c in range(N_CORES)
        ]

    _RUNNER = runner
    return runner


def run(e_seq, q_param, Wk, Wv, Wo, bo, trace=False):
    in_maps = make_in_maps(e_seq, q_param, Wk, Wv, Wo, bo)
    runner = _get_runner()
    results = runner(in_maps)
    out = np.concatenate([results[c]["out"] for c in range(N_CORES)], axis=0)
    return out, results


def kernel(e_seq, q_param, Wk, Wv, Wo, bo):
    out, _ = run(e_seq, q_param, Wk, Wv, Wo, bo)
    return out.astype(np.float32)


# revision 19
# speedup vs baseline: 180.7786x; 180.7786x over previous
"""AttnWindowPool Trainium2 kernel.

Math (per output row t, window w = t-3..t, per head h):
  k = e @ Wk, v = e @ Wv                       [L, H*128]
  s_h[t'] = q_h . k[t', h, :]                  (logits, shared across the 4 offsets)
  att = softmax over the window of s  ->  pooled = sum_w att_w * v[t-3+w]
Softmax-without-max identity (logits are O(5), exp can't overflow in fp32):
  pooled[t] = (sum_{s=t-3..t} E[s] * v[s]) / (sum_{s=t-3..t} E[s]),  E = exp(scale*s)
which turns the windowed attention into ONE constant banded matmul over
U[s] = E[s]*v[s] (plus the same banded sum of E for the denominator).
  out = pooled @ Wo + bo

Sharding: L split across 8 cores (2048 rows each) + one extra 128-row "halo"
block in front (previous core's last rows; zeros + E-mask for core 0).
All matmuls run as float32r (FP22 single-pass mode).
"""

import sys

sys.path.insert(0, "/opt/trn_rl_repo")

from contextlib import ExitStack

import numpy as np

import concourse.bass as bass
import concourse.tile as tile
from concourse import mybir
from concourse.masks import make_identity

# ---- problem constants (hardcoded per the grading contract) ----
L, D_IN, D_OUT, H, DH = 16384, 2048, 2048, 16, 128
W_LEFT = 3
N_CORES = 8
LC = L // N_CORES          # 2048 own rows per core
LH = LC + 128              # + one 128-row halo block in front
TB = LH // 128             # 17 t-blocks (block 0 = halo)
KC = D_IN // 128           # 16 contraction chunks
NC_ = D_OUT // 128         # 16 feature chunks
SCALE = 1.0 / float(np.sqrt(DH))
F32 = mybir.dt.float32
F32R = mybir.dt.float32r
# free-dim panels over the LH columns (matmul moving-operand max 512 fp32)
PANELS = [(0, 512), (512, 512), (1024, 512), (1536, 512), (2048, 128)]
NP4 = 4                    # 512-wide output panels over D_OUT
SPLIT_WAITS = True         # HW codegen needs 1-wait instrs; CoreSim can't parse NoOps


def _band_consts():
    d = np.zeros((128, 128), np.float32)   # diagT[s,t] = 1 if 0 <= t-s <= 3
    c = np.zeros((128, 128), np.float32)   # cornT[s,t] = 1 if s-t >= 125
    s = np.arange(128)[:, None]
    t = np.arange(128)[None, :]
    d[(t - s >= 0) & (t - s <= 3)] = 1.0
    c[(s - t) >= 125] = 1.0
    return d, c


def build_nc():
    nc = bass.Bass("TRN2", target_bir_lowering=False, debug=False)

    e_t = nc.dram_tensor("e", [LH, D_IN], F32R, kind="ExternalInput")
    emask_t = nc.dram_tensor("emask", [LH], F32, kind="ExternalInput")
    wqf_t = nc.dram_tensor("wqf", [D_IN, H], F32R, kind="ExternalInput")
    wv_t = nc.dram_tensor("wv", [D_IN, D_OUT], F32R, kind="ExternalInput")
    wo_t = nc.dram_tensor("wo", [D_OUT, D_OUT], F32R, kind="ExternalInput")
    bo_t = nc.dram_tensor("bo", [D_OUT], F32, kind="ExternalInput")
    out_t = nc.dram_tensor("out", [LC, D_OUT], F32, kind="ExternalOutput")

    vnat_t = nc.dram_tensor("vnat", [LH, D_OUT], F32R)  # scratch

    diag_np, corn_np = _band_consts()
    diag_d = nc.inline_tensor(diag_np, "band_diag")
    corn_d = nc.inline_tensor(corn_np, "band_corn")

    with tile.TileContext(nc) as tc:
        _kernel_body(
            tc, e_t, emask_t, wqf_t, wv_t, wo_t, bo_t, out_t, vnat_t,
            diag_d, corn_d,
        )
    if SPLIT_WAITS:
        _split_matmul_waits(nc)
    return nc


def _split_matmul_waits(nc):
    """walrus's LDWEIGHTS struct only has room for one sync-wait command, so
    a Matmult carrying >1 on_wait fails codegen ("Too many sync wait
    commands").  Move the extra waits onto NoOps just before the matmul on
    the PE queue (one wait per NoOp)."""
    for func in nc.m.functions:
        for blk in func.blocks:
            new_insts = []
            for inst in blk.instructions:
                si = getattr(inst, "sync_info", None)
                if (
                    si is not None
                    and si.on_wait
                    and len(si.on_wait) > 1
                    and not isinstance(inst, mybir.InstNoOp)
                ):
                    waits = list(si.on_wait)
                    for w in waits[:-1]:
                        nop = mybir.InstNoOp(
                            name=nc.get_next_instruction_name(),
                            ins=[],
                            outs=[],
                            sync_info=mybir.SyncInfo(on_wait=[w], on_update=[]),
                            bass_nofuse=True,
                            engine=inst.engine,
                        )
                        new_insts.append(nop)
                    inst.sync_info = mybir.SyncInfo(
                        on_wait=[waits[-1]], on_update=list(si.on_update)
                    )
                new_insts.append(inst)
            blk.instructions[:] = new_insts


def _kernel_body(tc, e_t, emask_t, wqf_t, wv_t, wo_t, bo_t, out_t, vnat_t,
                 diag_d, corn_d):
    nc = tc.nc
    R = lambda ap: ap.bitcast(F32R)

    with ExitStack() as top:
        # pools that live for the whole kernel
        singles = top.enter_context(tc.tile_pool(name="singles", bufs=1))
        ps_acc = top.enter_context(tc.tile_pool(name="ps_acc", bufs=1, space="PSUM"))
        ps_tr = top.enter_context(tc.tile_pool(name="ps_tr", bufs=2, space="PSUM"))

        ident0 = singles.tile([128, 128], F32)
        make_identity(nc, ident0)
        ident = singles.tile([128, 128], F32R)
        nc.vector.tensor_copy(ident[:], ident0[:])
        diag_sb = singles.tile([128, 128], F32R)
        nc.sync.dma_start(diag_sb[:], diag_d.ap().bitcast(F32R))
        corn_sb = singles.tile([128, 128], F32R)
        nc.sync.dma_start(corn_sb[:], corn_d.ap().bitcast(F32R))

        # folded logit weights Wk @ blockdiag(q): [D_IN, H]
        wqf_sb = singles.tile([128, KC, H], F32R)
        nc.sync.dma_start(
            wqf_sb[:], wqf_t.ap().rearrange("(a p) h -> p a h", p=128))

        emask_sb = singles.tile([128, TB], F32)
        nc.sync.dma_start(emask_sb[:], emask_t.ap().rearrange("(a p) -> p a", p=128))

        # E16 natural [128 t, tb, h]
        e16 = singles.tile([128, TB, H], F32)

        # ---------------- Phase A: e -> eT (PE transpose) ----------------
        pab = top.enter_context(ExitStack())
        with ExitStack() as pa:
            eT_pool = pab.enter_context(tc.tile_pool(name="eT", bufs=1))
            eblk_pool = pa.enter_context(tc.tile_pool(name="eblk", bufs=3))
            eT = eT_pool.tile([128, KC, LH], F32R)
            for tb in range(TB):
                eb = eblk_pool.tile([128, D_IN], F32R, tag="eblk")
                nc.sync.dma_start(eb[:], e_t[tb * 128 : (tb + 1) * 128, :])
                for kc in range(KC):
                    tp = ps_tr.tile([128, 128], F32R, tag="tr")
                    nc.tensor.transpose(R(tp[:]), R(eb[:, kc * 128 : (kc + 1) * 128]), R(ident[:]))
                    dst = eT[:, kc, tb * 128 : (tb + 1) * 128]
                    if (tb * KC + kc) % 2 == 0:
                        nc.vector.tensor_copy(dst, tp[:])
                    else:
                        nc.scalar.copy(dst, tp[:])

        # ---- Phase A2: logits S16[t, h] = eT.T @ wqf, E = exp(scale*S) ----
        for tb in range(TB):
            sps = ps_acc.tile([128, H], F32, tag="s16", name="sps")
            for kc in range(KC):
                nc.tensor.matmul(
                    sps[:], R(eT[:, kc, tb * 128 : (tb + 1) * 128]), R(wqf_sb[:, kc, :]),
                    start=(kc == 0), stop=(kc == KC - 1),
                )
            nc.scalar.activation(e16[:, tb, :], sps[:],
                                 mybir.ActivationFunctionType.Exp, scale=SCALE)
        nc.vector.tensor_tensor(
            e16[:], e16[:],
            emask_sb[:, :, None].to_broadcast((128, TB, H)), mybir.AluOpType.mult,
        )

        # ------------- Phase B: V projection (weights streamed once) -------------
        with ExitStack() as pb:
            wsl_pool = pb.enter_context(tc.tile_pool(name="wsl", bufs=2))
            kv_pool = pb.enter_context(tc.tile_pool(name="kv", bufs=2))
            vb_pool = pb.enter_context(tc.tile_pool(name="vb", bufs=4))
            for c in range(NC_):
                # --- V chunk: VT_c = Wv[:, c].T @ eT  -> transpose -> vnat dram
                wv_sl = wsl_pool.tile([128, KC, 128], F32R, tag="w")
                nc.sync.dma_start(
                    wv_sl[:],
                    wv_t[:, c * 128 : (c + 1) * 128].rearrange("(a p) n -> p a n", p=128),
                )
                vtc = kv_pool.tile([128, LH], F32R, tag="kv")
                accs = [
                    ps_acc.tile([128, w], F32, tag=f"acc{i}", name=f"acc{i}")
                    for i, (o, w) in enumerate(PANELS)
                ]
                for kc in range(KC):
                    for i, (o, w) in enumerate(PANELS):
                        nc.tensor.matmul(
                            accs[i][:], R(wv_sl[:, kc, :]), R(eT[:, kc, o : o + w]),
                            start=(kc == 0), stop=(kc == KC - 1),
                        )
                for i, (o, w) in enumerate(PANELS):
                    if i % 2 == 0:
                        nc.vector.tensor_copy(vtc[:, o : o + w], accs[i][:])
                    else:
                        nc.scalar.copy(vtc[:, o : o + w], accs[i][:])
                for tb in range(TB):
                    tp = ps_tr.tile([128, 128], F32R, tag="tr")
                    nc.tensor.transpose(R(tp[:]), R(vtc[:, tb * 128 : (tb + 1) * 128]), R(ident[:]))
                    vb = vb_pool.tile([128, 128], F32R, tag="vb")
                    if tb % 2 == 0:
                        nc.vector.tensor_copy(vb[:], tp[:])
                    else:
                        nc.scalar.copy(vb[:], tp[:])
                    nc.sync.dma_start(
                        vnat_t[tb * 128 : (tb + 1) * 128, c * 128 : (c + 1) * 128], vb[:]
                    )

        pab.close()  # frees eT before pooledT is allocated

        # ------------- Phase C: U = E*V, banded sums, divide, transpose -------------
        pooledT_pool = top.enter_context(tc.tile_pool(name="pooledT", bufs=1))
        pooledT = pooledT_pool.tile([128, NC_, LC], F32R)
        with ExitStack() as pc:
            vblk_pool = pc.enter_context(tc.tile_pool(name="vblk", bufs=3))
            u_pool = pc.enter_context(tc.tile_pool(name="u", bufs=2))
            pl_pool = pc.enter_context(tc.tile_pool(name="pl", bufs=2))
            sm_pool = pc.enter_context(tc.tile_pool(name="sm", bufs=2))
            u_prev = None
            for tb in range(TB):
                vb = vblk_pool.tile([128, D_OUT], F32R, tag="vblk")
                nc.sync.dma_start(vb[:], vnat_t[tb * 128 : (tb + 1) * 128, :])
                u = u_pool.tile([128, D_OUT + H], F32R, tag="u")
                nc.vector.tensor_tensor(
                    u[:, :D_OUT].rearrange("p (h d) -> p h d", d=DH),
                    vb[:].rearrange("p (h d) -> p h d", d=DH),
                    e16[:, tb, :, None].to_broadcast((128, H, DH)),
                    mybir.AluOpType.mult,
                )
                nc.scalar.copy(u[:, D_OUT:], e16[:, tb, :])
                if tb >= 1:
                    t_own = tb - 1  # output t-block index (0..15)
                    prs = []
                    for i in range(NP4):
                        pr = ps_acc.tile([128, 512], F32, tag=f"acc{i}")
                        nc.tensor.matmul(pr[:], R(diag_sb[:]), R(u[:, i * 512 : (i + 1) * 512]),
                                         start=True, stop=False)
                        nc.tensor.matmul(pr[:], R(corn_sb[:]),
                                         R(u_prev[:, i * 512 : (i + 1) * 512]),
                                         start=False, stop=True)
                        prs.append(pr)
                    prd = ps_acc.tile([128, H], F32, tag="acc4")
                    nc.tensor.matmul(prd[:], R(diag_sb[:]), R(u[:, D_OUT:]),
                                     start=True, stop=False)
                    nc.tensor.matmul(prd[:], R(corn_sb[:]), R(u_prev[:, D_OUT:]),
                                     start=False, stop=True)
                    dinv = sm_pool.tile([128, H], F32, tag="dinv")
                    nc.vector.reciprocal(dinv[:], prd[:])
                    pooled = pl_pool.tile([128, D_OUT], F32R, tag="pl")
                    for i in range(NP4):
                        nc.vector.tensor_tensor(
                            pooled[:, i * 512 : (i + 1) * 512].rearrange(
                                "p (h d) -> p h d", d=DH),
                            prs[i][:].rearrange("p (h d) -> p h d", d=DH),
                            dinv[:, i * 4 : (i + 1) * 4, None].to_broadcast(
                                (128, 4, DH)),
                            mybir.AluOpType.mult,
                        )
                    for hc in range(NC_):
                        tp = ps_tr.tile([128, 128], F32R, tag="tr")
                        nc.tensor.transpose(
                            R(tp[:]), R(pooled[:, hc * 128 : (hc + 1) * 128]), R(ident[:]))
                        dst = pooledT[:, hc, t_own * 128 : (t_own + 1) * 128]
                        if hc % 2 == 0:
                            nc.vector.tensor_copy(dst, tp[:])
                        else:
                            nc.scalar.copy(dst, tp[:])
                u_prev = u

        # ------------- Phase D: out = pooledT.T @ Wo + bo -------------
        with ExitStack() as pd:
            wo_pool = pd.enter_context(tc.tile_pool(name="wo", bufs=2))
            ob_pool = pd.enter_context(tc.tile_pool(name="ob", bufs=4))
            bo_pool = pd.enter_context(tc.tile_pool(name="bo", bufs=1))
            bo_sb = bo_pool.tile([128, D_OUT], F32)
            bo_bc = bass.AP(tensor=bo_t, offset=0, ap=[[0, 128], [1, D_OUT]])
            nc.gpsimd.dma_start(bo_sb[:], bo_bc)
            PW = 256
            for i in range(D_OUT // PW):
                wo_sl = wo_pool.tile([128, KC, PW], F32R, tag="wo")
                nc.sync.dma_start(
                    wo_sl[:],
                    wo_t[:, i * PW : (i + 1) * PW].rearrange(
                        "(a p) n -> p a n", p=128),
                )
                for tb in range(TB - 1):
                    ops = ps_acc.tile([128, PW], F32, tag=f"acc{tb % 4}",
                                      name="ops")
                    for hc in range(NC_):
                        nc.tensor.matmul(
                            ops[:], R(pooledT[:, hc, tb * 128 : (tb + 1) * 128]),
                            R(wo_sl[:, hc, :]),
                            start=(hc == 0), stop=(hc == NC_ - 1),
                        )
                    osb = ob_pool.tile([128, PW], F32, tag="ob")
                    nc.vector.tensor_tensor(
                        osb[:], ops[:], bo_sb[:, i * PW : (i + 1) * PW],
                        mybir.AluOpType.add,
                    )
                    nc.sync.dma_start(
                        out_t[tb * 128 : (tb + 1) * 128, i * PW : (i + 1) * PW],
                        osb[:],
                    )


_NC_CACHE = None


def _get_nc():
    global _NC_CACHE
    if _NC_CACHE is None:
        _NC_CACHE = build_nc()
    return _NC_CACHE


def make_in_maps(e_seq, q_param, Wk, Wv, Wo, bo):
    e_seq = np.ascontiguousarray(e_seq, np.float32)
    # fold Wk and q into the per-head logit weights (weight preprocessing):
    # s_h[t] = q_h . (e[t] @ Wk)[h*128:(h+1)*128] = e[t] . wqf[:, h]
    wqf = np.einsum(
        "khd,hd->kh",
        np.asarray(Wk, np.float32).reshape(D_IN, H, DH),
        np.asarray(q_param, np.float32),
    ).astype(np.float32)
    in_maps = []
    for c in range(N_CORES):
        s = c * LC
        if c == 0:
            eh = np.concatenate(
                [np.zeros((128, D_IN), np.float32), e_seq[:LC]], axis=0)
            em = np.ones(LH, np.float32)
            em[:128] = 0.0
        else:
            eh = e_seq[s - 128 : s + LC]
            em = np.ones(LH, np.float32)
        in_maps.append({
            "e": np.ascontiguousarray(eh),
            "emask": em,
            "wqf": wqf,
            "wv": np.ascontiguousarray(Wv, np.float32),
            "wo": np.ascontiguousarray(Wo, np.float32),
            "bo": np.ascontiguousarray(bo, np.float32),
        })
    return in_maps


_RUNNER = None


def _get_runner():
    """Cached jitted 8-core runner (mirrors bass2jax.run_bass_via_pjrt)."""
    global _RUNNER
    if _RUNNER is not None:
        return _RUNNER
    import jax
    from jax.sharding import Mesh, PartitionSpec
    from jax.experimental.shard_map import shard_map
    from concourse import mybir as mb
    from concourse.bass2jax import (
        _bass_exec_p, install_neuronx_cc_hook, partition_id_tensor,
    )

    install_neuronx_cc_hook()
    nc = _get_nc()
    partition_name = (
        nc.partition_id_tensor.name if nc.partition_id_tensor else None
    )
    in_names, out_names, out_avals, zero_shapes = [], [], [], []
    for alloc in nc.m.functions[0].allocations:
        if not isinstance(alloc, mb.MemoryLocationSet):
            continue
        name = alloc.memorylocations[0].name
        if alloc.kind == "ExternalInput":
            if name != partition_name:
                in_names.append(name)
        elif alloc.kind == "ExternalOutput":
            out_names.append(name)
            shape = tuple(alloc.tensor_shape)
            dtype = mb.dt.np(alloc.dtype)
            out_avals.append(jax.core.ShapedArray(shape, dtype))
            zero_shapes.append((shape, dtype))
    n_params = len(in_names)
    n_outs = len(out_avals)
    all_names = in_names + out_names
    if partition_name is not None:
        all_names = all_names + [partition_name]
    donate = tuple(range(n_params, n_params + n_outs))

    def _body(*args):
        operands = list(args)
        if partition_name is not None:
            operands.append(partition_id_tensor())
        outs = _bass_exec_p.bind(
            *operands,
            out_avals=tuple(out_avals),
            in_names=tuple(all_names),
            out_names=tuple(out_names),
            lowering_input_output_aliases=(),
            sim_require_finite=True,
            sim_require_nnan=True,
            nc=nc,
        )
        return tuple(outs)

    devices = jax.devices()[:N_CORES]
    mesh = Mesh(np.asarray(devices), ("core",))
    sharded = jax.jit(
        shard_map(_body, mesh=mesh,
                  in_specs=(PartitionSpec("core"),) * (n_params + n_outs),
                  out_specs=(PartitionSpec("core"),) * n_outs,
                  check_rep=False),
        donate_argnums=donate, keep_unused=True,
    )

    from jax.sharding import NamedSharding

    shard = NamedSharding(mesh, PartitionSpec("core"))
    mk_zeros = jax.jit(
        lambda: tuple(
            jax.numpy.zeros((N_CORES * s[0], *s[1:]), d) for (s, d) in zero_shapes
        ),
        out_shardings=(shard,) * n_outs,
    )

    def place(in_maps):
        per_core = [[np.asarray(m[n]) for n in in_names] for m in in_maps]
        concat_in = [
            np.concatenate([per_core[c][i] for c in range(N_CORES)], axis=0)
            for i in range(n_params)
        ]
        return [jax.device_put(a, shard) for a in concat_in]

    def exec_placed(d_in, pull=True):
        out_arrs = sharded(*d_in, *mk_zeros())
        jax.block_until_ready(out_arrs)
        if not pull:
            return None
        out_np = [np.asarray(a) for a in out_arrs]
        return [
            {n: out_np[i].reshape(N_CORES, *zero_shapes[i][0])[c]
             for i, n in enumerate(out_names)}
            for c in range(N_CORES)
        ]

    def runner(in_maps):
        return exec_placed(place(in_maps))

    runner.place = place
    runner.exec_placed = exec_placed
    _RUNNER = runner
    return runner


def run(e_seq, q_param, Wk, Wv, Wo, bo, trace=False):
    in_maps = make_in_maps(e_seq, q_param, Wk, Wv, Wo, bo)
    runner = _get_runner()
    results = runner(in_maps)
    out = np.concatenate([results[c]["out"] for c in range(N_CORES)], axis=0)
    return out, results


def kernel(e_seq, q_param, Wk, Wv, Wo, bo):
    out, _ = run(e_seq, q_param, Wk, Wv, Wo, bo)
    return out.astype(np.float32)
